# revision 1
# baseline (speedup 1.0000x reference)
"""Cosine-attention Trainium2 kernel (nn_CosineAttention_54082228191953).

Sharding: 8 NeuronCores, one attention head per core (tensor-parallel on H;
B=2 batches handled per core). Each core computes qkv projection for its head,
cosine attention with per-head positional bias, and a partial output
projection (attn_out_h @ w_out[64h:64h+64]); the host sums the 8 partials.

Shapes (hardcoded): B=2, N=2048, C=512, H=8, D=64.

On-device layout: everything transposed (head-dim / contraction-dim on
partitions) so PE matmuls stream at 1 cycle/row using float32r:
  S^T[j,i] accumulated in PSUM on top of an identity-matmul seed of
  pos_bias^T (f16, host-transposed), exp on ScalarE, and
  attn_out^T = [V | 1].T @ P^T which yields softmax denominators for free
  in row 64 of the augmented output.
"""
import sys

sys.path.insert(0, "/opt/trn_rl_repo")

import numpy as np
from contextlib import ExitStack

import concourse.bass as bass
from concourse import bacc
import concourse.mybir as mybir
import concourse.tile as tile
from concourse.bass_utils import run_bass_kernel_spmd
from concourse.masks import make_identity

H, D, B, N, C = 8, 64, 2, 2048, 512
IC = 2            # i-chunks
ICW = N // IC     # 1024 i per chunk
JT = N // 128     # 16 j tiles
F32, F32R, F16 = mybir.dt.float32, mybir.dt.float32r, mybir.dt.float16

TRACE = False          # set by test.py for profiling runs
LAST_RESULTS = None    # BassKernelResults of the last run


def _build(t_val: float):
    """Build the single-core SPMD program (same program on all 8 cores)."""
    nc = bacc.Bacc("TRN2", target_bir_lowering=False, debug=False)

    xT_d = nc.dram_tensor("xT", [B, C, N], F32R, kind="ExternalInput").ap()
    wq_d = nc.dram_tensor("wq", [C, D], F32R, kind="ExternalInput").ap()
    wk_d = nc.dram_tensor("wk", [C, D], F32R, kind="ExternalInput").ap()
    wv_d = nc.dram_tensor("wv", [C, D], F32R, kind="ExternalInput").ap()
    wo_d = nc.dram_tensor("wo", [D, C], F32R, kind="ExternalInput").ap()
    biasT_d = nc.dram_tensor("biasT", [N, N], F16, kind="ExternalInput").ap()
    pout_d = nc.dram_tensor("pout", [B, N, C], F32, kind="ExternalOutput").ap()

    scratch = nc.dram_tensor("scratch", [8, N], F32).ap()  # rinv bounce rows

    with tile.TileContext(nc) as tc, ExitStack() as ctx:
        persist = ctx.enter_context(tc.tile_pool(name="persist", bufs=1))
        work = ctx.enter_context(tc.tile_pool(name="work", bufs=2))
        xtp = ctx.enter_context(tc.tile_pool(name="xtp", bufs=1))
        small = ctx.enter_context(tc.tile_pool(name="small", bufs=1))
        biasp = ctx.enter_context(tc.tile_pool(name="biasp", bufs=6))
        ptp = ctx.enter_context(tc.tile_pool(name="ptp", bufs=3))
        outp = ctx.enter_context(tc.tile_pool(name="outp", bufs=4))
        ps = ctx.enter_context(tc.tile_pool(name="ps", bufs=1, space="PSUM"))

        # ---- constants
        ident128 = persist.tile([128, 128], F16, tag="ident128")
        make_identity(nc, ident128)
        ident64 = persist.tile([64, 64], F32, tag="ident64")
        make_identity(nc, ident64)
        ones64 = persist.tile([64, 1], F32R, tag="ones64")
        nc.vector.memset(ones64.bitcast(F32), 1.0)

        # ---- weights
        wq_s = persist.tile([128, 4, D], F32R, tag="wq")
        wk_s = persist.tile([128, 4, D], F32R, tag="wk")
        wv_s = persist.tile([128, 4, D], F32R, tag="wv")
        for cc in range(4):
            nc.sync.dma_start(out=wq_s[:, cc, :], in_=wq_d[cc * 128:(cc + 1) * 128, :])
            nc.sync.dma_start(out=wk_s[:, cc, :], in_=wk_d[cc * 128:(cc + 1) * 128, :])
            nc.sync.dma_start(out=wv_s[:, cc, :], in_=wv_d[cc * 128:(cc + 1) * 128, :])
        wo_s = persist.tile([D, C], F32R, tag="wo")
        nc.sync.dma_start(out=wo_s, in_=wo_d)

        # ---- phase A: projections + l2-normalize (both batches)
        qhat = [persist.tile([D, N], F32R, tag=f"qhat{b}", name=f"qhat{b}") for b in range(B)]
        khat = [persist.tile([D, N], F32R, tag=f"khat{b}", name=f"khat{b}") for b in range(B)]
        # v in [j, d] layout + ones column, per j-tile: [128, JT*(D+1)]
        vaug = [persist.tile([128, JT * (D + 1)], F32R, tag=f"vaug{b}",
                             name=f"vaug{b}") for b in range(B)]

        for b in range(B):
            xt = [xtp.tile([128, N], F32R, tag=f"xt{cc}", name=f"xt{cc}") for cc in range(4)]
            for cc in range(4):
                nc.sync.dma_start(out=xt[cc], in_=xT_d[b, cc * 128:(cc + 1) * 128, :])

            nc.vector.memset(vaug[b].bitcast(F32), 1.0)

            for ti, (w_s, dst, scale) in enumerate([
                (wq_s, qhat[b], 1.0 / (t_val * t_val)),
                (wk_s, khat[b], 1.0),
                (wv_s, None, None),
            ]):
                raw = work.tile([D, N], F32, tag="raw")
                for half in range(2):
                    pt = ps.tile([D, ICW], F32, tag=f"st{half}", name="pt")
                    for cc in range(4):
                        for f in range(2):
                            sl = slice(half * ICW + f * 512, half * ICW + (f + 1) * 512)
                            nc.tensor.matmul(pt[:, f * 512:(f + 1) * 512],
                                             w_s[:, cc, :], xt[cc][:, sl],
                                             start=(cc == 0), stop=(cc == 3))
                    nc.vector.tensor_copy(raw[:, half * ICW:(half + 1) * ICW], pt)

                if dst is None:
                    # v: transpose [d, j] -> [j, d] per j-tile into vaug
                    for jt in range(JT):
                        vtr = ps.tile([128, D], F32, tag="st1")
                        nc.tensor.transpose(
                            vtr, raw[:, jt * 128:(jt + 1) * 128], ident64)
                        nc.vector.tensor_copy(
                            vaug[b][:, jt * (D + 1):jt * (D + 1) + D], vtr)
                    continue

                # q/k: rinv = scale_fn / ||row||, folded t via Sqrt(x/t^2)
                sq = small.tile([D, N], F32R, tag="sq")
                nc.vector.tensor_mul(sq, raw, raw)
                rt = small.tile([1, N], F32, tag="rt")
                for half in range(2):
                    sp = ps.tile([1, ICW], F32, tag="oa0")
                    for f in range(2):
                        sl = slice(half * ICW + f * 512, half * ICW + (f + 1) * 512)
                        nc.tensor.matmul(sp[:, f * 512:(f + 1) * 512],
                                         ones64, sq[:, sl], start=True, stop=True)
                    nc.scalar.activation(
                        out=rt[:, half * ICW:(half + 1) * ICW], in_=sp,
                        func=mybir.ActivationFunctionType.Sqrt, scale=scale)
                rinv = small.tile([1, N], F32, tag="rinv")
                nc.vector.reciprocal(rinv, rt)
                srow = scratch[b * 2 + ti:b * 2 + ti + 1, :]
                nc.sync.dma_start(out=srow, in_=rinv)
                rbc = small.tile([D, N], F32, tag="rbc")
                nc.sync.dma_start(out=rbc, in_=srow.partition_broadcast(D))
                nc.vector.tensor_mul(dst, raw, rbc)

        # ---- phase B: attention + partial out-projection
        for ic in range(2):
            i0 = ic * ICW
            oa = [ps.tile([D + 1, ICW], F32, tag=f"oa{b}", name=f"oa{b}") for b in range(B)]
            for jt in range(JT):
                bt = biasp.tile([128, ICW], F16, tag="bias")
                nc.sync.dma_start(
                    out=bt, in_=biasT_d[jt * 128:(jt + 1) * 128, i0:i0 + ICW])
                for b in range(B):
                    st = ps.tile([128, ICW], F32, tag=f"st{b}")
                    for f in range(2):
                        nc.tensor.matmul(st[:, f * 512:(f + 1) * 512], ident128,
                                         bt[:, f * 512:(f + 1) * 512],
                                         start=True, stop=False,
                                         skip_group_check=True)
                    for f in range(2):
                        nc.tensor.matmul(
                            st[:, f * 512:(f + 1) * 512],
                            khat[b][:, jt * 128:(jt + 1) * 128],
                            qhat[b][:, i0 + f * 512:i0 + (f + 1) * 512],
                            start=False, stop=True, skip_group_check=True)
                    pt = ptp.tile([128, ICW], F32R, tag=f"pt{b}")
                    nc.scalar.activation(out=pt, in_=st,
                                         func=mybir.ActivationFunctionType.Exp)
                    for f in range(2):
                        nc.tensor.matmul(
                            oa[b][:, f * 512:(f + 1) * 512],
                            vaug[b][:, jt * (D + 1):(jt + 1) * (D + 1)],
                            pt[:, f * 512:(f + 1) * 512],
                            start=(jt == 0), stop=(jt == JT - 1),
                            skip_group_check=True)

            for b in range(B):
                rsinv = small.tile([1, ICW], F32, tag="rsinv")
                nc.vector.reciprocal(rsinv, oa[b][D:D + 1, :])
                attnT = small.tile([D, ICW], F32, tag="attnT")
                nc.vector.tensor_copy(attnT, oa[b][0:D, :])
                srow = scratch[4 + ic * 2 + b:4 + ic * 2 + b + 1, 0:ICW]
                nc.sync.dma_start(out=srow, in_=rsinv)
                rsbc = small.tile([D, ICW], F32, tag="rsbc")
                nc.sync.dma_start(out=rsbc, in_=srow.partition_broadcast(D))
                attnTn = work.tile([D, ICW], F32R, tag="attnTn")
                nc.vector.tensor_mul(attnTn, attnT, rsbc)
                for nt in range(ICW // 128):
                    pp = ps.tile([128, C], F32, tag=f"st{b}")
                    nc.tensor.matmul(pp, attnTn[:, nt * 128:(nt + 1) * 128],
                                     wo_s, start=True, stop=True)
                    ot = outp.tile([128, C], F32, tag="ot")
                    nc.vector.tensor_copy(ot, pp)
                    r0 = i0 + nt * 128
                    nc.sync.dma_start(out=pout_d[b, r0:r0 + 128, :], in_=ot)

    nc.compile()
    return nc


def _run_device(x, w_qkv, w_out, pos_bias, t_val):
    global LAST_RESULTS
    nc = _build(t_val)

    x = np.asarray(x, dtype=np.float32)
    w_qkv = np.asarray(w_qkv, dtype=np.float32)
    w_out = np.asarray(w_out, dtype=np.float32)
    pos_bias = np.asarray(pos_bias, dtype=np.float32)

    xT = np.ascontiguousarray(x.transpose(0, 2, 1))  # [B, C, N]
    w3 = w_qkv.reshape(C, H, D, 3)
    in_maps = []
    for h in range(H):
        in_maps.append({
            "xT": xT,
            "wq": np.ascontiguousarray(w3[:, h, :, 0]),
            "wk": np.ascontiguousarray(w3[:, h, :, 1]),
            "wv": np.ascontiguousarray(w3[:, h, :, 2]),
            "wo": np.ascontiguousarray(w_out[h * D:(h + 1) * D, :]),
            "biasT": np.ascontiguousarray(pos_bias[h].T).astype(np.float16),
        })

    res = run_bass_kernel_spmd(nc, in_maps, list(range(H)), trace=TRACE)
    LAST_RESULTS = res
    acc = np.zeros((B, N, C), dtype=np.float64)
    for h in range(H):
        acc += res.results[h]["pout"]
    return acc.astype(np.float32)


def _reference_numpy(x, w_qkv, w_out, pos_bias, temperature, mask):
    """Exact-math fallback (used only when mask has padded positions)."""
    x = np.asarray(x, dtype=np.float32)
    qkv = (x @ np.asarray(w_qkv)).reshape(B, N, H, D, 3)
    qkv = np.transpose(qkv, (4, 0, 2, 1, 3))
    q, k, v = qkv[0], qkv[1], qkv[2]

    def l2n(t):
        n = np.linalg.norm(t, axis=-1, keepdims=True)
        return t / np.maximum(n, 1e-12)

    q, k = l2n(q), l2n(k)
    dots = np.einsum("bhid,bhjd->bhij", q, k) * np.float32(temperature)
    dots = dots + np.asarray(pos_bias)[None]
    valid = ~np.asarray(mask)
    am = ~(valid[:, None, :, None] & valid[:, None, None, :])
    dots = np.where(am, -np.finfo(np.float32).max, dots)
    dots = dots - dots.max(axis=-1, keepdims=True)
    e = np.exp(dots)
    attn = e / e.sum(axis=-1, keepdims=True)
    out = np.einsum("bhij,bhjd->bhid", attn, v)
    out = np.transpose(out, (0, 2, 1, 3)).reshape(B, N, H * D)
    return (out @ np.asarray(w_out)).astype(np.float32)


def kernel(x, w_qkv, w_out, pos_bias, temperature, mask):
    mask = np.asarray(mask)
    t_val = float(np.asarray(temperature))
    if mask.any():
        return _reference_numpy(x, w_qkv, w_out, pos_bias, t_val, mask)
    return _run_device(x, w_qkv, w_out, pos_bias, t_val)



# revision 40
# speedup vs baseline: 1.6799x; 1.6799x over previous
"""Cosine-attention Trainium2 kernel (nn_CosineAttention_54082228191953).

Sharding: 8 NeuronCores, one attention head per core (tensor-parallel on H;
B=2 batches per core). Each core computes the qkv projection for its head,
cosine attention with per-head positional bias, and a partial output
projection (attn_num_h @ w_out[64h:64h+64]) plus the softmax denominator
row; the host divides by the denominator and sums the 8 head partials.

Shapes (hardcoded): B=2, N=2048, C=512, H=8, D=64.

On-device layout: transposed (head-dim / contraction-dim on partitions) so
PE matmuls stream at 1 cycle/row:
  - q|k projected stacked on 128 partitions in one matmul chain; row norms
    via a ones-selector matmul; reciprocal broadcast back across partitions
    with a tiny selector matmul (no DRAM bounce).
  - v projected directly into [j, d] layout (x^T chunk as stationary).
  - S^T[j,i] accumulated in PSUM on top of a pos_bias^T seed done with an
    fp8 DoubleRow identity matmul (0.5 cycles/column; second k-slot zero).
  - exp on ScalarE (bf16 out), attn_num^T = [V | 1].T @ P^T gives the
    denominator for free in row 64; AV matmuls are software-pipelined one
    j-step behind exp to keep PE from head-of-line blocking.
  - out projection emits UNNORMALIZED partials; host multiplies by 1/den.
"""
import sys

sys.path.insert(0, "/opt/trn_rl_repo")

import numpy as np
from contextlib import ExitStack

import ml_dtypes

import concourse.bass as bass
from concourse import bacc
import concourse.mybir as mybir
import concourse.tile as tile
from concourse.bass_utils import run_bass_kernel_spmd
from concourse.masks import make_identity

H, D, B, N, C = 8, 64, 2, 2048, 512
IC = 2            # i-chunks
ICW = N // IC     # 1024 i per chunk
JT = N // 128     # 16 j tiles
F32, F32R = mybir.dt.float32, mybir.dt.float32r
F16, BF16 = mybir.dt.float16, mybir.dt.bfloat16
F8 = mybir.dt.float8e4
DR = mybir.MatmulPerfMode.DoubleRow
EXPF = mybir.ActivationFunctionType.Exp
SQRTF = mybir.ActivationFunctionType.Sqrt

TRACE = False          # set by test.py for profiling runs
LAST_RESULTS = None    # BassKernelResults of the last run


def _build(t_val: float):
    """Build the single-core SPMD program (same program on all 8 cores)."""
    nc = bacc.Bacc("TRN2", target_bir_lowering=False, debug=False)

    # host-packed layouts to minimize DMA count (HWDGE issue is ~630ns each):
    #   xh[b, p, cc, n] = x[b, n, cc*128+p]
    #   wall[:, cc*192:cc*192+128] = w_qk chunk cc; [:, cc*192+128:(cc+1)*192]
    #     = w_v chunk cc; [0:64, 768:1280] = w_out
    #   pout[b, ic, p, nt, c] -> out row i0 + nt*128 + p
    #   den rows are (ic, b) pairs: k = ic*2 + b
    xh_d = nc.dram_tensor("xh", [B, 128, 2, 4, ICW], BF16, kind="ExternalInput").ap()
    wall_d = nc.dram_tensor("wall", [128, 4 * 192 + C], BF16, kind="ExternalInput").ap()
    # packed f32 constants: [:, 0:2] ssq selector, [0:2, 2] sqrt scales,
    # [0:2, 4:132] broadcast selector
    consts_d = nc.dram_tensor("consts", [128, 132], F32R, kind="ExternalInput").ap()
    identdr_d = nc.dram_tensor("identdr", [128, 256], F8, kind="ExternalInput").ap()
    biasT_d = nc.dram_tensor("biasT", [N, N], F8, kind="ExternalInput").ap()
    pout_d = nc.dram_tensor("pout", [B, IC, 128, ICW // 128, C], BF16,
                            kind="ExternalOutput").ap()
    den_d = nc.dram_tensor("den", [128, ICW], F32, kind="ExternalOutput").ap()

    with tile.TileContext(nc) as tc, ExitStack() as ctx:
        persist = ctx.enter_context(tc.tile_pool(name="persist", bufs=1))
        xtp = ctx.enter_context(tc.tile_pool(name="xtp", bufs=2))
        work = ctx.enter_context(tc.tile_pool(name="work", bufs=2))
        small = ctx.enter_context(tc.tile_pool(name="small", bufs=2))
        biasp = ctx.enter_context(tc.tile_pool(name="biasp", bufs=1))
        ptp = ctx.enter_context(tc.tile_pool(name="ptp", bufs=3))
        outp = ctx.enter_context(tc.tile_pool(name="outp", bufs=2))
        ps = ctx.enter_context(tc.tile_pool(name="ps", bufs=1, space="PSUM"))

        # ---- constants (host-packed; see consts_d layout)
        consts = persist.tile([128, 132], F32R, tag="consts")
        nc.sync.dma_start(out=consts, in_=consts_d)
        ident_dr = persist.tile([128, 2, 128], F8, tag="identdr")
        nc.sync.dma_start(out=ident_dr, in_=identdr_d)
        ones2r = consts[:, 0:2]
        sel2r = consts[0:2, 4:132]
        tsc = consts.bitcast(F32)[0:2, 2:3]
        # absorb the Sqrt act-table load while DMAs stream in
        warm = persist.tile([2, 1], F32, tag="warm")
        nc.scalar.activation(out=warm, in_=tsc, func=SQRTF)

        # ---- weights: one DMA for everything
        wall = persist.tile([128, 4 * 192 + C], BF16, tag="wall")
        nc.sync.dma_start(out=wall, in_=wall_d)
        wqk_cc = [wall[:, cc * 192:cc * 192 + 128] for cc in range(4)]
        wv_cc = [wall[:, cc * 192 + 128:(cc + 1) * 192] for cc in range(4)]
        wo_s = wall[0:D, 768:768 + C]

        # bias tiles: 6-deep manual rotation; [:, 1, :] stays zero so the
        # DoubleRow seed's second k-slot contributes nothing.
        bts = []
        for k in range(6):
            bt = biasp.tile([128, 2, ICW], F8, tag=f"bt{k}", name=f"bt{k}")
            nc.gpsimd.memset(bt[:, 1, :], 0.0)
            bts.append(bt)

        # denominator rows (b, ic) at partitions 32*(ic*2+b) (engine ops
        # need quadrant-aligned partition bases)
        dall = persist.tile([128, ICW], F32, tag="dall")
        qhat = [persist.tile([D, N], F32R, tag=f"qhat{b}", name=f"qhat{b}") for b in range(B)]
        khat = [persist.tile([D, N], F32R, tag=f"khat{b}", name=f"khat{b}") for b in range(B)]
        # v in [j, d] layout + ones column, per j-tile: [128, JT*(D+1)]
        vaug = [persist.tile([128, JT * (D + 1)], BF16, tag=f"vaug{b}",
                             name=f"vaug{b}") for b in range(B)]

        # ---- phase A: projections + l2-normalize. Per-batch chains so b0's
        # normalization completes while b1's projection still streams; the
        # v-projection fills PE while ACT works the sqrt chain.
        xts, qkraws, sqs, rts, rrs = [], [], [], [], []
        for b in range(B):
            xt = xtp.tile([128, 4, N], BF16, tag="xt", name=f"xt{b}")
            # (half, cc) pieces: first projection starts after 1/8 of the load
            for half in range(2):
                for cc in range(4):
                    nc.sync.dma_start(out=xt[:, cc, half * ICW:(half + 1) * ICW],
                                      in_=xh_d[b, :, half, cc, :])
            xts.append(xt)
            nc.gpsimd.memset(vaug[b], 1.0)
            qkraws.append(work.tile([128, N], F32, tag="qkraw", name=f"qkraw{b}"))
            sqs.append(work.tile([128, N], F32R, tag="sq", name=f"sq{b}"))
            rts.append(small.tile([2, N], F32, tag=f"rt{b}", name=f"rt{b}", bufs=1))
            rrs.append(small.tile([2, N], F32R, tag=f"rr{b}", name=f"rr{b}", bufs=1))

        for b in range(B):
            # projections: DVE stashes raw q|k, ACT squares (both read PSUM)
            for half in range(2):
                hs = slice(half * ICW, (half + 1) * ICW)
                pj = ps.tile([128, ICW], F32, tag=f"st{half}", name="pj")
                for f in range(2):
                    fs = slice(f * 512, (f + 1) * 512)
                    gs = slice(half * ICW + f * 512, half * ICW + (f + 1) * 512)
                    for cc in range(4):
                        nc.tensor.matmul(pj[:, fs], wqk_cc[cc], xts[b][:, cc, gs],
                                         start=(cc == 0), stop=(cc == 3),
                                         skip_group_check=True)
                nc.vector.tensor_copy(qkraws[b][:, hs], pj)
                nc.scalar.square(sqs[b][:, hs], pj)
        # row norms + normalize, half-major: attention step jt only needs
        # khat columns jt*128..(jt+1)*128, so the h1 chains can trail while
        # phase B already runs on h0.
        for half in range(2):
            hs = slice(half * ICW, (half + 1) * ICW)
            for b in range(B):
                ssq = ps.tile([2, ICW], F32, tag=f"oa{half}", name="ssq")
                for f in range(2):
                    fs = slice(f * 512, (f + 1) * 512)
                    gs = slice(half * ICW + f * 512, half * ICW + (f + 1) * 512)
                    nc.tensor.matmul(ssq[:, fs], ones2r, sqs[b][:, gs],
                                     start=True, stop=True,
                                     skip_group_check=True)
                nc.scalar.activation(out=rts[b][:, hs], in_=ssq, func=SQRTF,
                                     scale=tsc[:, 0:1])
            for b in range(B):
                with nc.allow_low_precision(reason="f32r holds full f32 bits"):
                    nc.vector.reciprocal(rrs[b][:, hs], rts[b][:, hs])
                rbc = ps.tile([128, ICW], F32, tag=f"oa{half}", name="rbc")
                for f in range(2):
                    fs = slice(f * 512, (f + 1) * 512)
                    gs = slice(half * ICW + f * 512, half * ICW + (f + 1) * 512)
                    nc.tensor.matmul(rbc[:, fs], sel2r, rrs[b][:, gs],
                                     start=True, stop=True,
                                     skip_group_check=True)
                nc.vector.tensor_mul(qhat[b][:, hs],
                                     qkraws[b][0:64, hs], rbc[0:64, :])
                nc.vector.tensor_mul(khat[b][:, hs],
                                     qkraws[b][64:128, hs], rbc[64:128, :])
        # absorb the Exp act-table load before the attention stream begins
        nc.scalar.activation(out=warm, in_=tsc, func=EXPF)
        # v directly in [j, d] layout; vaug chunk jt is only needed at
        # attention step jt, so this sits late without costing anything
        for b in range(B):
            for jt in range(JT):
                vt = ps.tile([128, D], F32, tag=f"oa{jt % 2}", name="vt")
                for cc in range(4):
                    nc.tensor.matmul(vt, xts[b][:, cc, jt * 128:(jt + 1) * 128],
                                     wv_cc[cc],
                                     start=(cc == 0), stop=(cc == 3))
                nc.vector.tensor_copy(vaug[b][:, jt * (D + 1):jt * (D + 1) + D], vt)

        # ---- phase B: attention; out-projection deferred to the end
        post = []  # deferred out-projection work: (b, ic, attnT, dnr)
        for ic in range(IC):
            i0 = ic * ICW
            oa = [ps.tile([D + 1, ICW], F32, tag=f"oa{b}", name=f"oa{b}") for b in range(B)]
            pend = []  # AV matmuls deferred one j-step: (b, jt, pt)
            for jt in range(JT):
                bt = bts[(ic * JT + jt) % 6]
                nc.sync.dma_start(
                    out=bt[:, 0, :], in_=biasT_d[jt * 128:(jt + 1) * 128, i0:i0 + ICW])
                for b in range(B):
                    # flush the deferred AV for this batch first (keeps PE
                    # from stalling on the freshly-issued exp)
                    if pend and pend[0][0] == b and pend[0][1] == jt - 1:
                        b2, jt2, pt2 = pend.pop(0)
                        for f in range(2):
                            nc.tensor.matmul(
                                oa[b2][:, f * 512:(f + 1) * 512],
                                vaug[b2][:, jt2 * (D + 1):(jt2 + 1) * (D + 1)],
                                pt2[:, f * 512:(f + 1) * 512],
                                start=(jt2 == 0), stop=(jt2 == JT - 1),
                                skip_group_check=True)
                    st = ps.tile([128, ICW], F32, tag=f"st{b}", name=f"st{b}")
                    for f in range(2):
                        nc.tensor.matmul(st[:, f * 512:(f + 1) * 512], ident_dr,
                                         bt[:, :, f * 512:(f + 1) * 512],
                                         start=True, stop=False, perf_mode=DR,
                                         skip_group_check=True)
                    for f in range(2):
                        nc.tensor.matmul(
                            st[:, f * 512:(f + 1) * 512],
                            khat[b][:, jt * 128:(jt + 1) * 128],
                            qhat[b][:, i0 + f * 512:i0 + (f + 1) * 512],
                            start=False, stop=True, skip_group_check=True)
                    pt = ptp.tile([128, ICW], BF16, tag=f"pt{b}", name=f"pt{b}")
                    nc.scalar.activation(out=pt, in_=st, func=EXPF)
                    pend.append((b, jt, pt))
            for b2, jt2, pt2 in pend:
                for f in range(2):
                    nc.tensor.matmul(
                        oa[b2][:, f * 512:(f + 1) * 512],
                        vaug[b2][:, jt2 * (D + 1):(jt2 + 1) * (D + 1)],
                        pt2[:, f * 512:(f + 1) * 512],
                        start=(jt2 == 0), stop=(jt2 == JT - 1),
                        skip_group_check=True)

            for b in range(B):
                attnT = small.tile([D, ICW], BF16, tag=f"attnT{ic}{b}",
                                   name=f"attnT{ic}{b}", bufs=1)
                if b == 0:
                    nc.vector.tensor_copy(attnT, oa[b][0:D, :])
                else:
                    nc.scalar.copy(attnT, oa[b][0:D, :])
                kk = 32 * (ic * 2 + b)
                nc.vector.tensor_copy(dall[kk:kk + 1, :], oa[b][D:D + 1, :])
                post.append((b, ic, attnT))

        # ---- out-projection tail (unnormalized; host divides by den).
        # PSUM->SBUF copies alternate between ScalarE and VectorE so neither
        # engine rate-limits the pq ping-pong; one batched store per (b, ic).
        nc.sync.dma_start(out=den_d, in_=dall)
        # all 4 PSUM tags are free now: round-robin pq across them, store
        # per pair so the last DMA trails the last copy by one pair only
        ptags = ["st0", "st1", "oa0", "oa1"]
        nq = 0
        for k, (b, ic, attnT) in enumerate(post):
            ot = outp.tile([128, ICW // 128, C], BF16, tag="ot")
            for pr in range(ICW // 256):
                pq = ps.tile([128, 2, C], F32, tag=ptags[nq % 4], name="pq")
                for h2 in range(2):
                    nt = pr * 2 + h2
                    nc.tensor.matmul(pq[:, h2, :], attnT[:, nt * 128:(nt + 1) * 128],
                                     wo_s, start=True, stop=True,
                                     skip_group_check=True)
                if nq % 2 == 0:
                    nc.vector.tensor_copy(ot[:, pr * 2:pr * 2 + 2, :], pq)
                else:
                    nc.scalar.copy(ot[:, pr * 2:pr * 2 + 2, :], pq)
                nc.sync.dma_start(out=pout_d[b, ic, :, pr * 2:pr * 2 + 2, :],
                                  in_=ot[:, pr * 2:pr * 2 + 2, :])
                nq += 1

    nc.compile()
    return nc


def _run_device(x, w_qkv, w_out, pos_bias, t_val):
    global LAST_RESULTS
    nc = _build(t_val)

    x = np.asarray(x, dtype=np.float32)
    w_qkv = np.asarray(w_qkv, dtype=np.float32)
    w_out = np.asarray(w_out, dtype=np.float32)
    pos_bias = np.asarray(pos_bias, dtype=np.float32)

    bf16 = ml_dtypes.bfloat16
    fp8 = ml_dtypes.float8_e4m3
    consts = np.zeros((128, 132), dtype=np.float32)
    consts[0:64, 0] = 1.0          # ones2 col 0: sum q rows
    consts[64:128, 1] = 1.0        # ones2 col 1: sum k rows
    consts[0, 2] = 1.0 / (t_val * t_val)  # sqrt scale, q row
    consts[1, 2] = 1.0                    # sqrt scale, k row
    consts[0, 4:68] = 1.0          # sel2 row 0 -> partitions 0-63
    consts[1, 68:132] = 1.0        # sel2 row 1 -> partitions 64-127
    identdr = np.zeros((128, 256), dtype=fp8)
    identdr[:, 0:128] = np.eye(128, dtype=np.float32).astype(fp8)
    # xh[b, p, half, cc, i] = x[b, half*ICW + i, cc*128 + p]
    xT = x.transpose(0, 2, 1)                                 # [B, C, N]
    xT = xT.reshape(B, 4, 128, 2, ICW)                        # [B, cc, p, half, i]
    xh = np.ascontiguousarray(xT.transpose(0, 2, 3, 1, 4)).astype(bf16)
    w3 = w_qkv.reshape(C, H, D, 3)
    in_maps = []
    for h in range(H):
        wall = np.zeros((128, 4 * 192 + C), dtype=np.float32)
        for cc in range(4):
            rows = slice(cc * 128, (cc + 1) * 128)
            wall[:, cc * 192:cc * 192 + D] = w3[rows, h, :, 0]
            wall[:, cc * 192 + D:cc * 192 + 128] = w3[rows, h, :, 1]
            wall[:, cc * 192 + 128:(cc + 1) * 192] = w3[rows, h, :, 2]
        wall[0:D, 768:768 + C] = w_out[h * D:(h + 1) * D, :]
        in_maps.append({
            "xh": xh,
            "wall": wall.astype(bf16),
            "biasT": np.ascontiguousarray(pos_bias[h].T).astype(fp8),
            "consts": consts,
            "identdr": identdr,
        })

    res = run_bass_kernel_spmd(nc, in_maps, list(range(H)), trace=TRACE)
    LAST_RESULTS = res
    acc = np.zeros((B, N, C), dtype=np.float64)
    for h in range(H):
        # pout[b, ic, p, nt, c] -> row i0 + nt*128 + p; den row k = ic*2 + b
        pout = np.asarray(res.results[h]["pout"], dtype=np.float64)
        pout = pout.transpose(0, 1, 3, 2, 4).reshape(B, N, C)
        den = np.asarray(res.results[h]["den"], dtype=np.float64)
        den = den[[0, 32, 64, 96]].reshape(IC, B, ICW).transpose(1, 0, 2).reshape(B, N)
        acc += pout / den[:, :, None]
    return acc.astype(np.float32)


def _reference_numpy(x, w_qkv, w_out, pos_bias, temperature, mask):
    """Exact-math fallback (used only when mask has padded positions)."""
    x = np.asarray(x, dtype=np.float32)
    qkv = (x @ np.asarray(w_qkv)).reshape(B, N, H, D, 3)
    qkv = np.transpose(qkv, (4, 0, 2, 1, 3))
    q, k, v = qkv[0], qkv[1], qkv[2]

    def l2n(t):
        n = np.linalg.norm(t, axis=-1, keepdims=True)
        return t / np.maximum(n, 1e-12)

    q, k = l2n(q), l2n(k)
    dots = np.einsum("bhid,bhjd->bhij", q, k) * np.float32(temperature)
    dots = dots + np.asarray(pos_bias)[None]
    valid = ~np.asarray(mask)
    am = ~(valid[:, None, :, None] & valid[:, None, None, :])
    dots = np.where(am, -np.finfo(np.float32).max, dots)
    dots = dots - dots.max(axis=-1, keepdims=True)
    e = np.exp(dots)
    attn = e / e.sum(axis=-1, keepdims=True)
    out = np.einsum("bhij,bhjd->bhid", attn, v)
    out = np.transpose(out, (0, 2, 1, 3)).reshape(B, N, H * D)
    return (out @ np.asarray(w_out)).astype(np.float32)


def kernel(x, w_qkv, w_out, pos_bias, temperature, mask):
    mask = np.asarray(mask)
    t_val = float(np.asarray(temperature))
    if mask.any():
        return _reference_numpy(x, w_qkv, w_out, pos_bias, t_val, mask)
    return _run_device(x, w_qkv, w_out, pos_bias, t_val)


# revision 43
# speedup vs baseline: 1.7015x; 1.0129x over previous
"""Cosine-attention Trainium2 kernel (nn_CosineAttention_54082228191953).

Sharding: 8 NeuronCores, one attention head per core (tensor-parallel on H;
B=2 batches per core). Each core computes the qkv projection for its head,
cosine attention with per-head positional bias, and a partial output
projection (attn_num_h @ w_out[64h:64h+64]) plus the softmax denominator
row; the host divides by the denominator and sums the 8 head partials.

Shapes (hardcoded): B=2, N=2048, C=512, H=8, D=64.

On-device layout: transposed (head-dim / contraction-dim on partitions) so
PE matmuls stream at 1 cycle/row:
  - q|k projected stacked on 128 partitions in one matmul chain; row norms
    via a ones-selector matmul; reciprocal broadcast back across partitions
    with a tiny selector matmul (no DRAM bounce).
  - v projected directly into [j, d] layout (x^T chunk as stationary).
  - S^T[j,i] accumulated in PSUM on top of a pos_bias^T seed done with an
    fp8 DoubleRow identity matmul (0.5 cycles/column; second k-slot zero).
  - exp on ScalarE (bf16 out), attn_num^T = [V | 1].T @ P^T gives the
    denominator for free in row 64; AV matmuls are software-pipelined one
    j-step behind exp to keep PE from head-of-line blocking.
  - out projection emits UNNORMALIZED partials; host multiplies by 1/den.
"""
import sys

sys.path.insert(0, "/opt/trn_rl_repo")

import numpy as np
from contextlib import ExitStack

import ml_dtypes

import concourse.bass as bass
from concourse import bacc
import concourse.mybir as mybir
import concourse.tile as tile
from concourse.bass_utils import run_bass_kernel_spmd
from concourse.masks import make_identity

H, D, B, N, C = 8, 64, 2, 2048, 512
IC = 2            # i-chunks
ICW = N // IC     # 1024 i per chunk
JT = N // 128     # 16 j tiles
F32, F32R = mybir.dt.float32, mybir.dt.float32r
F16, BF16 = mybir.dt.float16, mybir.dt.bfloat16
F8 = mybir.dt.float8e4
DR = mybir.MatmulPerfMode.DoubleRow
EXPF = mybir.ActivationFunctionType.Exp
SQRTF = mybir.ActivationFunctionType.Sqrt

TRACE = False          # set by test.py for profiling runs
LAST_RESULTS = None    # BassKernelResults of the last run


def _build(t_val: float):
    """Build the single-core SPMD program (same program on all 8 cores)."""
    nc = bacc.Bacc("TRN2", target_bir_lowering=False, debug=False)

    # host-packed layouts to minimize DMA count (HWDGE issue is ~630ns each):
    #   xh[b, p, cc, n] = x[b, n, cc*128+p]
    #   wall[:, cc*192:cc*192+128] = w_qk chunk cc; [:, cc*192+128:(cc+1)*192]
    #     = w_v chunk cc; [0:64, 768:1280] = w_out
    #   pout[b, ic, p, nt, c] -> out row i0 + nt*128 + p
    #   den rows are (ic, b) pairs: k = ic*2 + b
    xh_d = nc.dram_tensor("xh", [B, 128, 2, 4, ICW], BF16, kind="ExternalInput").ap()
    wall_d = nc.dram_tensor("wall", [128, 4 * 192 + C], BF16, kind="ExternalInput").ap()
    # packed f32 constants: [:, 0:2] ssq selector, [0:2, 2] sqrt scales,
    # [0:2, 4:132] broadcast selector
    consts_d = nc.dram_tensor("consts", [128, 132], F32R, kind="ExternalInput").ap()
    identdr_d = nc.dram_tensor("identdr", [128, 256], F8, kind="ExternalInput").ap()
    biasT_d = nc.dram_tensor("biasT", [N, N], F8, kind="ExternalInput").ap()
    pout_d = nc.dram_tensor("pout", [B, IC, 128, ICW // 128, C], BF16,
                            kind="ExternalOutput").ap()
    den_d = nc.dram_tensor("den", [128, ICW], F32, kind="ExternalOutput").ap()

    with tile.TileContext(nc) as tc, ExitStack() as ctx:
        persist = ctx.enter_context(tc.tile_pool(name="persist", bufs=1))
        xtp = ctx.enter_context(tc.tile_pool(name="xtp", bufs=2))
        work = ctx.enter_context(tc.tile_pool(name="work", bufs=2))
        small = ctx.enter_context(tc.tile_pool(name="small", bufs=2))
        biasp = ctx.enter_context(tc.tile_pool(name="biasp", bufs=1))
        ptp = ctx.enter_context(tc.tile_pool(name="ptp", bufs=4))
        outp = ctx.enter_context(tc.tile_pool(name="outp", bufs=2))
        ps = ctx.enter_context(tc.tile_pool(name="ps", bufs=1, space="PSUM"))

        # ---- constants (host-packed; see consts_d layout)
        consts = persist.tile([128, 132], F32R, tag="consts")
        nc.sync.dma_start(out=consts, in_=consts_d)
        ident_dr = persist.tile([128, 2, 128], F8, tag="identdr")
        nc.sync.dma_start(out=ident_dr, in_=identdr_d)
        ones2r = consts[:, 0:2]
        sel2r = consts[0:2, 4:132]
        tsc = consts.bitcast(F32)[0:2, 2:3]
        # absorb the Sqrt act-table load while DMAs stream in
        warm = persist.tile([2, 1], F32, tag="warm")
        nc.scalar.activation(out=warm, in_=tsc, func=SQRTF)

        # ---- weights: one DMA for everything
        wall = persist.tile([128, 4 * 192 + C], BF16, tag="wall")
        nc.sync.dma_start(out=wall, in_=wall_d)
        wqk_cc = [wall[:, cc * 192:cc * 192 + 128] for cc in range(4)]
        wv_cc = [wall[:, cc * 192 + 128:(cc + 1) * 192] for cc in range(4)]
        wo_s = wall[0:D, 768:768 + C]

        # bias tiles: 6-deep manual rotation; [:, 1, :] stays zero so the
        # DoubleRow seed's second k-slot contributes nothing.
        bts = []
        for k in range(6):
            bt = biasp.tile([128, 2, ICW], F8, tag=f"bt{k}", name=f"bt{k}")
            nc.gpsimd.memset(bt[:, 1, :], 0.0)
            bts.append(bt)

        # denominator rows (b, ic) at partitions 32*(ic*2+b) (engine ops
        # need quadrant-aligned partition bases)
        dall = persist.tile([128, ICW], F32, tag="dall")
        qhat = [persist.tile([D, N], F32R, tag=f"qhat{b}", name=f"qhat{b}") for b in range(B)]
        khat = [persist.tile([D, N], F32R, tag=f"khat{b}", name=f"khat{b}") for b in range(B)]
        # v in [j, d] layout + ones column, per j-tile: [128, JT*(D+1)]
        vaug = [persist.tile([128, JT * (D + 1)], BF16, tag=f"vaug{b}",
                             name=f"vaug{b}") for b in range(B)]

        # ---- phase A: projections + l2-normalize. Per-batch chains so b0's
        # normalization completes while b1's projection still streams; the
        # v-projection fills PE while ACT works the sqrt chain.
        xts, qkraws, sqs, rts, rrs = [], [], [], [], []
        for b in range(B):
            xts.append(xtp.tile([128, 4, N], BF16, tag="xt", name=f"xt{b}"))
            nc.gpsimd.memset(vaug[b], 1.0)
            qkraws.append(work.tile([128, N], F32, tag="qkraw", name=f"qkraw{b}"))
            sqs.append(work.tile([128, N], F32R, tag="sq", name=f"sq{b}"))
            rts.append(small.tile([2, N], F32, tag=f"rt{b}", name=f"rt{b}", bufs=1))
            rrs.append(small.tile([2, N], F32R, tag=f"rr{b}", name=f"rr{b}", bufs=1))
        # x pieces land half-major so BOTH batches' h0 chains finish early;
        # the h1 chains trail into phase B harmlessly
        for half in range(2):
            for b in range(B):
                for cc in range(4):
                    nc.sync.dma_start(
                        out=xts[b][:, cc, half * ICW:(half + 1) * ICW],
                        in_=xh_d[b, :, half, cc, :])

        for half in range(2):
            # projections: DVE stashes raw q|k, ACT squares (both read PSUM)
            for b in range(B):
                hs = slice(half * ICW, (half + 1) * ICW)
                pj = ps.tile([128, ICW], F32, tag=f"st{b}", name="pj")
                for f in range(2):
                    fs = slice(f * 512, (f + 1) * 512)
                    gs = slice(half * ICW + f * 512, half * ICW + (f + 1) * 512)
                    for cc in range(4):
                        nc.tensor.matmul(pj[:, fs], wqk_cc[cc], xts[b][:, cc, gs],
                                         start=(cc == 0), stop=(cc == 3),
                                         skip_group_check=True)
                nc.vector.tensor_copy(qkraws[b][:, hs], pj)
                nc.scalar.square(sqs[b][:, hs], pj)
        # row norms + normalize, half-major: attention step jt only needs
        # khat columns jt*128..(jt+1)*128, so the h1 chains can trail while
        # phase B already runs on h0.
        for half in range(2):
            hs = slice(half * ICW, (half + 1) * ICW)
            for b in range(B):
                ssq = ps.tile([2, ICW], F32, tag=f"oa{half}", name="ssq")
                for f in range(2):
                    fs = slice(f * 512, (f + 1) * 512)
                    gs = slice(half * ICW + f * 512, half * ICW + (f + 1) * 512)
                    nc.tensor.matmul(ssq[:, fs], ones2r, sqs[b][:, gs],
                                     start=True, stop=True,
                                     skip_group_check=True)
                nc.scalar.activation(out=rts[b][:, hs], in_=ssq, func=SQRTF,
                                     scale=tsc[:, 0:1])
            for b in range(B):
                with nc.allow_low_precision(reason="f32r holds full f32 bits"):
                    nc.vector.reciprocal(rrs[b][:, hs], rts[b][:, hs])
                rbc = ps.tile([128, ICW], F32, tag=f"oa{half}", name="rbc")
                for f in range(2):
                    fs = slice(f * 512, (f + 1) * 512)
                    gs = slice(half * ICW + f * 512, half * ICW + (f + 1) * 512)
                    nc.tensor.matmul(rbc[:, fs], sel2r, rrs[b][:, gs],
                                     start=True, stop=True,
                                     skip_group_check=True)
                nc.vector.tensor_mul(qhat[b][:, hs],
                                     qkraws[b][0:64, hs], rbc[0:64, :])
                nc.vector.tensor_mul(khat[b][:, hs],
                                     qkraws[b][64:128, hs], rbc[64:128, :])
        # absorb the Exp act-table load before the attention stream begins
        nc.scalar.activation(out=warm, in_=tsc, func=EXPF)
        # v directly in [j, d] layout; vaug chunk jt is only needed at
        # attention step jt, so this sits late without costing anything
        for b in range(B):
            for jt in range(JT):
                vt = ps.tile([128, D], F32, tag=f"oa{jt % 2}", name="vt")
                for cc in range(4):
                    nc.tensor.matmul(vt, xts[b][:, cc, jt * 128:(jt + 1) * 128],
                                     wv_cc[cc],
                                     start=(cc == 0), stop=(cc == 3))
                nc.vector.tensor_copy(vaug[b][:, jt * (D + 1):jt * (D + 1) + D], vt)

        # ---- phase B: attention; out-projection deferred to the end
        post = []  # deferred out-projection work: (b, ic, attnT, dnr)
        for ic in range(IC):
            i0 = ic * ICW
            oa = [ps.tile([D + 1, ICW], F32, tag=f"oa{b}", name=f"oa{b}") for b in range(B)]
            pend = []  # AV matmuls deferred one j-step: (b, jt, pt)
            for jt in range(JT):
                bt = bts[(ic * JT + jt) % 6]
                nc.sync.dma_start(
                    out=bt[:, 0, :], in_=biasT_d[jt * 128:(jt + 1) * 128, i0:i0 + ICW])
                for b in range(B):
                    st = ps.tile([128, ICW], F32, tag=f"st{b}", name=f"st{b}")
                    for f in range(2):
                        nc.tensor.matmul(st[:, f * 512:(f + 1) * 512], ident_dr,
                                         bt[:, :, f * 512:(f + 1) * 512],
                                         start=True, stop=False, perf_mode=DR,
                                         skip_group_check=True)
                    for f in range(2):
                        nc.tensor.matmul(
                            st[:, f * 512:(f + 1) * 512],
                            khat[b][:, jt * 128:(jt + 1) * 128],
                            qhat[b][:, i0 + f * 512:i0 + (f + 1) * 512],
                            start=False, stop=True, skip_group_check=True)
                    pt = ptp.tile([128, ICW], BF16, tag=f"pt{b}", name=f"pt{b}")
                    nc.scalar.activation(out=pt, in_=st, func=EXPF)
                    # flush this batch's deferred AV after its seed/S so the
                    # PE step stays dense while exp runs on ACT
                    if pend and pend[0][0] == b and pend[0][1] == jt - 1:
                        b2, jt2, pt2 = pend.pop(0)
                        for f in range(2):
                            nc.tensor.matmul(
                                oa[b2][:, f * 512:(f + 1) * 512],
                                vaug[b2][:, jt2 * (D + 1):(jt2 + 1) * (D + 1)],
                                pt2[:, f * 512:(f + 1) * 512],
                                start=(jt2 == 0), stop=(jt2 == JT - 1),
                                skip_group_check=True)
                    pend.append((b, jt, pt))
            for b2, jt2, pt2 in pend:
                for f in range(2):
                    nc.tensor.matmul(
                        oa[b2][:, f * 512:(f + 1) * 512],
                        vaug[b2][:, jt2 * (D + 1):(jt2 + 1) * (D + 1)],
                        pt2[:, f * 512:(f + 1) * 512],
                        start=(jt2 == 0), stop=(jt2 == JT - 1),
                        skip_group_check=True)

            for b in range(B):
                attnT = small.tile([D, ICW], BF16, tag=f"attnT{ic}{b}",
                                   name=f"attnT{ic}{b}", bufs=1)
                if b == 0:
                    nc.vector.tensor_copy(attnT, oa[b][0:D, :])
                else:
                    nc.scalar.copy(attnT, oa[b][0:D, :])
                kk = 32 * (ic * 2 + b)
                nc.vector.tensor_copy(dall[kk:kk + 1, :], oa[b][D:D + 1, :])
                post.append((b, ic, attnT))

        # ---- out-projection tail (unnormalized; host divides by den).
        # PSUM->SBUF copies alternate between ScalarE and VectorE so neither
        # engine rate-limits the pq ping-pong; one batched store per (b, ic).
        nc.sync.dma_start(out=den_d, in_=dall)
        # all 4 PSUM tags are free now: round-robin pq across them, store
        # per pair so the last DMA trails the last copy by one pair only
        ptags = ["st0", "st1", "oa0", "oa1"]
        nq = 0
        for k, (b, ic, attnT) in enumerate(post):
            ot = outp.tile([128, ICW // 128, C], BF16, tag="ot")
            for pr in range(ICW // 256):
                pq = ps.tile([128, 2, C], F32, tag=ptags[nq % 4], name="pq")
                for h2 in range(2):
                    nt = pr * 2 + h2
                    nc.tensor.matmul(pq[:, h2, :], attnT[:, nt * 128:(nt + 1) * 128],
                                     wo_s, start=True, stop=True,
                                     skip_group_check=True)
                if nq % 2 == 0:
                    nc.vector.tensor_copy(ot[:, pr * 2:pr * 2 + 2, :], pq)
                else:
                    nc.scalar.copy(ot[:, pr * 2:pr * 2 + 2, :], pq)
                nc.sync.dma_start(out=pout_d[b, ic, :, pr * 2:pr * 2 + 2, :],
                                  in_=ot[:, pr * 2:pr * 2 + 2, :])
                nq += 1

    nc.compile()
    return nc


def _run_device(x, w_qkv, w_out, pos_bias, t_val):
    global LAST_RESULTS
    nc = _build(t_val)

    x = np.asarray(x, dtype=np.float32)
    w_qkv = np.asarray(w_qkv, dtype=np.float32)
    w_out = np.asarray(w_out, dtype=np.float32)
    pos_bias = np.asarray(pos_bias, dtype=np.float32)

    bf16 = ml_dtypes.bfloat16
    fp8 = ml_dtypes.float8_e4m3
    consts = np.zeros((128, 132), dtype=np.float32)
    consts[0:64, 0] = 1.0          # ones2 col 0: sum q rows
    consts[64:128, 1] = 1.0        # ones2 col 1: sum k rows
    consts[0, 2] = 1.0 / (t_val * t_val)  # sqrt scale, q row
    consts[1, 2] = 1.0                    # sqrt scale, k row
    consts[0, 4:68] = 1.0          # sel2 row 0 -> partitions 0-63
    consts[1, 68:132] = 1.0        # sel2 row 1 -> partitions 64-127
    identdr = np.zeros((128, 256), dtype=fp8)
    identdr[:, 0:128] = np.eye(128, dtype=np.float32).astype(fp8)
    # xh[b, p, half, cc, i] = x[b, half*ICW + i, cc*128 + p]
    xT = x.transpose(0, 2, 1)                                 # [B, C, N]
    xT = xT.reshape(B, 4, 128, 2, ICW)                        # [B, cc, p, half, i]
    xh = np.ascontiguousarray(xT.transpose(0, 2, 3, 1, 4)).astype(bf16)
    w3 = w_qkv.reshape(C, H, D, 3)
    in_maps = []
    for h in range(H):
        wall = np.zeros((128, 4 * 192 + C), dtype=np.float32)
        for cc in range(4):
            rows = slice(cc * 128, (cc + 1) * 128)
            wall[:, cc * 192:cc * 192 + D] = w3[rows, h, :, 0]
            wall[:, cc * 192 + D:cc * 192 + 128] = w3[rows, h, :, 1]
            wall[:, cc * 192 + 128:(cc + 1) * 192] = w3[rows, h, :, 2]
        wall[0:D, 768:768 + C] = w_out[h * D:(h + 1) * D, :]
        in_maps.append({
            "xh": xh,
            "wall": wall.astype(bf16),
            "biasT": np.ascontiguousarray(pos_bias[h].T).astype(fp8),
            "consts": consts,
            "identdr": identdr,
        })

    res = run_bass_kernel_spmd(nc, in_maps, list(range(H)), trace=TRACE)
    LAST_RESULTS = res
    acc = np.zeros((B, N, C), dtype=np.float64)
    for h in range(H):
        # pout[b, ic, p, nt, c] -> row i0 + nt*128 + p; den row k = ic*2 + b
        pout = np.asarray(res.results[h]["pout"], dtype=np.float64)
        pout = pout.transpose(0, 1, 3, 2, 4).reshape(B, N, C)
        den = np.asarray(res.results[h]["den"], dtype=np.float64)
        den = den[[0, 32, 64, 96]].reshape(IC, B, ICW).transpose(1, 0, 2).reshape(B, N)
        acc += pout / den[:, :, None]
    return acc.astype(np.float32)


def _reference_numpy(x, w_qkv, w_out, pos_bias, temperature, mask):
    """Exact-math fallback (used only when mask has padded positions)."""
    x = np.asarray(x, dtype=np.float32)
    qkv = (x @ np.asarray(w_qkv)).reshape(B, N, H, D, 3)
    qkv = np.transpose(qkv, (4, 0, 2, 1, 3))
    q, k, v = qkv[0], qkv[1], qkv[2]

    def l2n(t):
        n = np.linalg.norm(t, axis=-1, keepdims=True)
        return t / np.maximum(n, 1e-12)

    q, k = l2n(q), l2n(k)
    dots = np.einsum("bhid,bhjd->bhij", q, k) * np.float32(temperature)
    dots = dots + np.asarray(pos_bias)[None]
    valid = ~np.asarray(mask)
    am = ~(valid[:, None, :, None] & valid[:, None, None, :])
    dots = np.where(am, -np.finfo(np.float32).max, dots)
    dots = dots - dots.max(axis=-1, keepdims=True)
    e = np.exp(dots)
    attn = e / e.sum(axis=-1, keepdims=True)
    out = np.einsum("bhij,bhjd->bhid", attn, v)
    out = np.transpose(out, (0, 2, 1, 3)).reshape(B, N, H * D)
    return (out @ np.asarray(w_out)).astype(np.float32)


def kernel(x, w_qkv, w_out, pos_bias, temperature, mask):
    mask = np.asarray(mask)
    t_val = float(np.asarray(temperature))
    if mask.any():
        return _reference_numpy(x, w_qkv, w_out, pos_bias, t_val, mask)
    return _run_device(x, w_qkv, w_out, pos_bias, t_val)
